# revision 9
# baseline (speedup 1.0000x reference)
"""Causal self-attention (B=4, T=2048, C=2048, H=16, HD=128) on 8 trn2 cores.

Sharding: core c handles batch b = c//2 and heads (c%2)*8 .. +8.
  - QKV projection column-sharded by head, attention head-sharded,
    c_proj row-sharded; the pair partial sums are combined on host.

v2: QKV projections run as 3-term fp8e4 DoubleRow matmuls (x = xh+xl,
W*32 = Wh+Wl, dropping the xl*Wl term; ~1e-3 rel err at 0.75x the fp32r
cost). Attention and c_proj run in bf16. All intermediates (q/k/v/O)
stay resident in SBUF -- no DRAM spills. Softmax denominator comes from
bf16 pair-sums of exp tiles, halving the ones-matmul count.

Self-contained: hardcodes shapes; builds one SPMD Bass program and runs
it on cores 0-7 via run_bass_kernel_spmd.
"""
import math

import numpy as np
import ml_dtypes

import concourse.bass as bass
import concourse.mybir as mybir
import concourse.tile as tile
from concourse.bass_utils import run_bass_kernel_spmd

F32 = mybir.dt.float32
F32R = mybir.dt.float32r
BF16 = mybir.dt.bfloat16
F8 = mybir.dt.float8e4
AF = mybir.ActivationFunctionType
ALU = mybir.AluOpType
DR = mybir.MatmulPerfMode.DoubleRow

E4 = ml_dtypes.float8_e4m3
BF = ml_dtypes.bfloat16

# problem dims
B, T, C, H = 4, 2048, 2048, 16
HD = 128
NCORES = 8
NH = H // 2          # heads per core
MCH = 512            # matmul moving-operand chunk
WSCALE = 32.0        # weight pre-scale before fp8 split

_ctr = [0]


def _legalize_waits(nc, max_waits=1):
    """This walrus build rejects >1 sync wait per instruction. Hoist extra
    waits onto same-engine NoOps inserted directly before the instruction."""
    n_split = 0
    for f in nc.m.functions:
        for blk in f.blocks:
            newil = []
            changed = False
            for inst in blk.instructions:
                si = inst.sync_info
                if si is not None and si.on_wait and len(si.on_wait) > max_waits:
                    waits = list(si.on_wait)
                    for w in waits[:-max_waits]:
                        _ctr[0] += 1
                        nop = mybir.InstNoOp(name=f"I-waitfix-{_ctr[0]}")
                        nop.engine = inst.engine
                        nop.sync_info = mybir.SyncInfo(on_wait=[w], on_update=[])
                        newil.append(nop)
                    inst.sync_info = mybir.SyncInfo(
                        on_wait=waits[-max_waits:], on_update=list(si.on_update)
                    )
                    changed = True
                    n_split += 1
                newil.append(inst)
            if changed:
                blk.instructions = newil
    return n_split


def build_program(T=T, C=C, NH=NH, use_bqkv=False, qtile=512, legalize=True):
    """One core's program: full pipeline for (1 batch, NH heads)."""
    CB2 = C // 256         # fp8 DoubleRow contraction groups
    TBn = T // 128         # token blocks
    QTILE = min(qtile, T)  # flash tq tile
    NQT = T // QTILE
    JMAX = QTILE // 128
    DV = NH * 128          # v/proj-shard width
    inv_sqrt_hd = 1.0 / math.sqrt(HD)
    inv_ws = 1.0 / WSCALE

    nc = bass.Bass()
    xh_d = nc.dram_tensor("xh", [CB2, 128, 2, T], F8, kind="ExternalInput")
    xl_d = nc.dram_tensor("xl", [CB2, 128, 2, T], F8, kind="ExternalInput")
    wqkh_d = nc.dram_tensor("wqkh", [2, NH, 128, CB2, 2, 128], F8, kind="ExternalInput")
    wqkl_d = nc.dram_tensor("wqkl", [2, NH, 128, CB2, 2, 128], F8, kind="ExternalInput")
    wvh_d = nc.dram_tensor("wvh", [CB2, 128, 2, DV], F8, kind="ExternalInput")
    wvl_d = nc.dram_tensor("wvl", [CB2, 128, 2, DV], F8, kind="ExternalInput")
    wp_d = nc.dram_tensor("wp", [NH, 128, C], BF16, kind="ExternalInput")
    cos2_d = nc.dram_tensor("cos2", [128, T], BF16, kind="ExternalInput")
    sin2s_d = nc.dram_tensor("sin2s", [128, T], BF16, kind="ExternalInput")
    mask_d = nc.dram_tensor("maskbig", [128, 2 * QTILE - 128], BF16, kind="ExternalInput")
    ones_d = nc.dram_tensor("ones128", [128, 128], BF16, kind="ExternalInput")
    if use_bqkv:
        bqk_d = nc.dram_tensor("bqk", [128, 2 * NH], F32, kind="ExternalInput")
        onecol_d = nc.dram_tensor("onecol", [1, 128], BF16, kind="ExternalInput")
        bv_d = nc.dram_tensor("bv", [1, DV], BF16, kind="ExternalInput")
    out_d = nc.dram_tensor("out_partial", [T, C], BF16, kind="ExternalOutput")

    with tile.TileContext(nc) as tc:
        # ---- long-lived SBUF residents --------------------------------
        qkres_cm = tc.tile_pool(name="qkres", bufs=1)
        qkres = qkres_cm.__enter__()
        vres_cm = tc.tile_pool(name="vres", bufs=1)
        vres = vres_cm.__enter__()
        xpool_cm = tc.tile_pool(name="xpool", bufs=1)
        xpool = xpool_cm.__enter__()

        # interleave x and wv loads per contraction group so phase A1 can
        # start after the first group lands (j-outer accumulation below)
        xhs, xls = [], []
        with tc.tile_pool(name="wvpool", bufs=1) as wvpool:
            wvhs, wvls = [], []
            for j in range(CB2):
                xt = xpool.tile([128, 2, T], F8, tag=f"xh{j}")
                nc.sync.dma_start(out=xt[:], in_=xh_d[j])
                xhs.append(xt)
                xt = xpool.tile([128, 2, T], F8, tag=f"xl{j}")
                nc.sync.dma_start(out=xt[:], in_=xl_d[j])
                xls.append(xt)
                wvt = wvpool.tile([128, 2, DV], F8, tag=f"wvh{j}")
                nc.sync.dma_start(out=wvt[:], in_=wvh_d[j])
                wvhs.append(wvt)
                wvt = wvpool.tile([128, 2, DV], F8, tag=f"wvl{j}")
                nc.sync.dma_start(out=wvt[:], in_=wvl_d[j])
                wvls.append(wvt)
            if use_bqkv:
                onecol = wvpool.tile([1, 128], BF16)
                nc.sync.dma_start(out=onecol[:], in_=onecol_d[:])
                bv = wvpool.tile([1, DV], BF16)
                nc.sync.dma_start(out=bv[:], in_=bv_d[:])
            vts = [vres.tile([128, DV], BF16, name=f"v{tb}", tag=f"v{tb}")
                   for tb in range(TBn)]
            qrs = [qkres.tile([128, T], BF16, name=f"qr{h}", tag=f"qr{h}")
                   for h in range(NH)]
            krs = [qkres.tile([128, T], BF16, name=f"kr{h}", tag=f"kr{h}")
                   for h in range(NH)]

            # ------------ Phase A1: V = x @ Wv (j-outer stripes) -------
            SW = min(4, TBn)            # stripe width: 4 tb x 2 banks
            with tc.tile_pool(name="psv", bufs=1, space="PSUM") as psvp:
                for st in range(0, TBn, SW):
                    psvs = [psvp.tile([128, DV], F32, tag=f"psv{i}")
                            for i in range(SW)]
                    for j in range(CB2):
                        for i in range(SW):
                            ts = slice((st + i) * 128, (st + i + 1) * 128)
                            for ws in (slice(0, 512), slice(512, DV)) if DV > 512 \
                                    else (slice(0, DV),):
                                nc.tensor.matmul(
                                    psvs[i][:, ws], xhs[j][:, :, ts],
                                    wvhs[j][:, :, ws],
                                    start=(j == 0), stop=False, perf_mode=DR)
                                nc.tensor.matmul(
                                    psvs[i][:, ws], xls[j][:, :, ts],
                                    wvhs[j][:, :, ws],
                                    start=False, stop=False, perf_mode=DR)
                                nc.tensor.matmul(
                                    psvs[i][:, ws], xhs[j][:, :, ts],
                                    wvls[j][:, :, ws],
                                    start=False,
                                    stop=(j == CB2 - 1 and not use_bqkv),
                                    perf_mode=DR)
                    for i in range(SW):
                        if use_bqkv:
                            for ws in (slice(0, 512), slice(512, DV)) if DV > 512 \
                                    else (slice(0, DV),):
                                nc.tensor.matmul(psvs[i][:, ws], onecol[:],
                                                 bv[:, ws], start=False, stop=True)
                        nc.scalar.activation(vts[st + i][:], psvs[i][:], AF.Copy,
                                             scale=inv_ws)

        # ------------- Phase A2: q^T, k^T per head + RoPE -------------
        with (
            tc.tile_pool(name="cspool", bufs=1) as cspool,
            tc.tile_pool(name="qepool", bufs=4) as qepool,
            tc.tile_pool(name="wqpool", bufs=2) as wqpool,
            tc.tile_pool(name="psq", bufs=2, space="PSUM") as psqp,
        ):
            cos2 = cspool.tile([128, T], BF16)
            nc.sync.dma_start(out=cos2[:], in_=cos2_d[:])
            sin2s = cspool.tile([128, T], BF16)
            nc.sync.dma_start(out=sin2s[:], in_=sin2s_d[:])
            if use_bqkv:
                bqk = cspool.tile([128, 2 * NH], F32)
                nc.sync.dma_start(out=bqk[:], in_=bqk_d[:])
            for s in range(2):
                dest = qrs if s == 0 else krs
                for h in range(NH):
                    wqh = wqpool.tile([128, CB2, 2, 128], F8, tag="wqh")
                    nc.sync.dma_start(out=wqh[:], in_=wqkh_d[s, h])
                    wql = wqpool.tile([128, CB2, 2, 128], F8, tag="wql")
                    nc.sync.dma_start(out=wql[:], in_=wqkl_d[s, h])
                    ps = psqp.tile([128, T], F32, tag="psq")
                    for t0 in range(0, T, MCH):
                        tsl = slice(t0, t0 + MCH)
                        for j in range(CB2):
                            nc.tensor.matmul(ps[:, tsl], wqh[:, j], xhs[j][:, :, tsl],
                                             start=(j == 0), stop=False, perf_mode=DR)
                            nc.tensor.matmul(ps[:, tsl], wqh[:, j], xls[j][:, :, tsl],
                                             start=False, stop=False, perf_mode=DR)
                            nc.tensor.matmul(ps[:, tsl], wql[:, j], xhs[j][:, :, tsl],
                                             start=False, stop=(j == CB2 - 1),
                                             perf_mode=DR)
                    hw2 = T // 2
                    for half in range(2):
                        hs = slice(half * hw2, (half + 1) * hw2)
                        qb = qepool.tile([128, hw2], BF16, tag="qb")
                        if use_bqkv:
                            # descale then add bias column on DVE (f32 path)
                            qf = qepool.tile([128, hw2], F32, tag="qf")
                            nc.scalar.activation(qf[:], ps[:, hs], AF.Copy,
                                                 scale=inv_ws)
                            nc.vector.tensor_scalar(
                                qb[:], qf[:], bqk[:, s * NH + h:s * NH + h + 1],
                                None, ALU.add)
                        else:
                            nc.scalar.activation(qb[:], ps[:, hs], AF.Copy,
                                                 scale=inv_ws)
                        qrot = qepool.tile([128, hw2], BF16, tag="qrot")
                        nc.sync.dma_start(out=qrot[0:64, :], in_=qb[64:128, :])
                        nc.sync.dma_start(out=qrot[64:128, :], in_=qb[0:64, :])
                        nc.vector.tensor_mul(qb[:], qb[:], cos2[:, hs])
                        nc.vector.tensor_mul(qrot[:], qrot[:], sin2s[:, hs])
                        nc.vector.tensor_add(dest[h][:, hs], qb[:], qrot[:])
        xpool_cm.__exit__(None, None, None)
        ohres_cm = tc.tile_pool(name="ohres", bufs=1)
        ohres = ohres_cm.__enter__()
        ohs = [ohres.tile([128, T], BF16, name=f"oh{h}", tag=f"oh{h}")
               for h in range(NH)]

        # ---------------- Phase B: causal flash attention ----------------
        with (
            tc.tile_pool(name="bcpool", bufs=1) as bcpool,
            tc.tile_pool(name="ppool", bufs=4) as ppool,
            tc.tile_pool(name="ropool", bufs=2) as ropool,
            tc.tile_pool(name="psS", bufs=2, space="PSUM") as psSp,
            tc.tile_pool(name="psO", bufs=2, space="PSUM") as psOp,
            tc.tile_pool(name="psR", bufs=2, space="PSUM") as psRp,
        ):
            maskt = bcpool.tile([128, 2 * QTILE - 128], BF16)
            nc.sync.dma_start(out=maskt[:], in_=mask_d[:])
            ones = bcpool.tile([128, 128], BF16)
            nc.sync.dma_start(out=ones[:], in_=ones_d[:])
            for h in range(NH):
                for qt in range(NQT):
                    ntk = (qt + 1) * JMAX
                    npair = ntk // 2
                    tqs = slice(qt * QTILE, (qt + 1) * QTILE)
                    psO = psOp.tile([128, QTILE], F32, tag="psO")
                    psR = psRp.tile([128, QTILE], F32, tag="psR")
                    for pr in range(npair):
                        tk0, tk1 = 2 * pr, 2 * pr + 1
                        psS = psSp.tile([128, 2 * QTILE], F32, tag="psS")
                        nc.tensor.matmul(
                            psS[:, 0:QTILE],
                            krs[h][:, tk0 * 128:(tk0 + 1) * 128], qrs[h][:, tqs],
                            start=True, stop=True)
                        nc.tensor.matmul(
                            psS[:, QTILE:2 * QTILE],
                            krs[h][:, tk1 * 128:(tk1 + 1) * 128], qrs[h][:, tqs],
                            start=True, stop=True)
                        pt = ppool.tile([128, 2 * QTILE], BF16, tag="pt")
                        nc.scalar.activation(pt[:], psS[:], AF.Exp,
                                             scale=inv_sqrt_hd)
                        for i, tk in enumerate((tk0, tk1)):
                            j = tk - qt * JMAX
                            if j >= 0:
                                m0 = (JMAX - 1 - j) * 128
                                nc.vector.tensor_mul(
                                    pt[:, i * QTILE:(i + 1) * QTILE],
                                    pt[:, i * QTILE:(i + 1) * QTILE],
                                    maskt[:, m0:m0 + QTILE])
                        nc.tensor.matmul(psO[:], vts[tk0][:, h * 128:(h + 1) * 128],
                                         pt[:, 0:QTILE],
                                         start=(pr == 0), stop=False)
                        nc.tensor.matmul(psO[:], vts[tk1][:, h * 128:(h + 1) * 128],
                                         pt[:, QTILE:2 * QTILE],
                                         start=False, stop=(pr == npair - 1))
                        pacc = ppool.tile([128, QTILE], BF16, tag="pacc")
                        nc.vector.tensor_add(pacc[:], pt[:, 0:QTILE],
                                             pt[:, QTILE:2 * QTILE])
                        nc.tensor.matmul(psR[:], ones[:], pacc[:],
                                         start=(pr == 0), stop=(pr == npair - 1))
                    rec = ropool.tile([128, QTILE], F32, tag="rec")
                    nc.vector.reciprocal(rec[:], psR[:])
                    nc.vector.tensor_mul(ohs[h][:, tqs], psO[:], rec[:])

        # ---------------- Phase C: out_partial = O @ Wp_shard ----------------
        with (
            tc.tile_pool(name="wppool", bufs=1) as wppool,
            tc.tile_pool(name="oepool", bufs=2) as oepool,
            tc.tile_pool(name="psP", bufs=2, space="PSUM") as psPp,
        ):
            wps = []
            for hd in range(NH):
                wpt = wppool.tile([128, C], BF16, tag=f"wp{hd}")
                nc.sync.dma_start(out=wpt[:], in_=wp_d[hd])
                wps.append(wpt)
            for tb in range(TBn):
                psP = psPp.tile([128, C], F32, tag="psP")
                for c0 in range(0, C, MCH):
                    for hd in range(NH):
                        nc.tensor.matmul(
                            psP[:, c0:c0 + MCH],
                            ohs[hd][:, tb * 128:(tb + 1) * 128],
                            wps[hd][:, c0:c0 + MCH],
                            start=(hd == 0), stop=(hd == NH - 1))
                outsb = oepool.tile([128, C], BF16, tag="outsb")
                nc.scalar.copy(out=outsb[:], in_=psP[:])
                nc.sync.dma_start(out=out_d[tb * 128:(tb + 1) * 128, :], in_=outsb[:])
        ohres_cm.__exit__(None, None, None)
        vres_cm.__exit__(None, None, None)
        qkres_cm.__exit__(None, None, None)

    if legalize:
        _legalize_waits(nc)
    return nc


# ---------------------------------------------------------------- host side

_PERM = np.concatenate([np.arange(0, HD, 2), np.arange(1, HD, 2)])  # de-interleave


def _split8(a):
    """fp8 residual split: a ~ hi + lo with hi, lo e4m3."""
    hi = a.astype(E4)
    lo = (a - hi.astype(np.float32)).astype(E4)
    return hi, lo


def shard_core(core, x, freqs_cos, freqs_sin, Wqkv, bqkv, Wproj,
               T=T, C=C, NH=NH, qtile=512, use_bqkv=False):
    """Build the in_map for one core."""
    CB2 = C // 256
    DV = NH * 128
    QTILE = min(qtile, T)
    b = core // 2
    hb = (core % 2) * NH

    # x^T packed for DoubleRow: [j, p, i, t] = x[t, j*256 + i*128 + p]
    xt = np.ascontiguousarray(
        x[b].T.reshape(CB2, 2, 128, T).transpose(0, 2, 1, 3))
    xh, xl = _split8(xt)

    # q/k weights: [2, NH, 128(m), CB2, 2, 128(p)] with RoPE de-interleave on m
    cols = (np.arange(2)[:, None, None] * C
            + (hb + np.arange(NH))[None, :, None] * HD + _PERM[None, None, :])
    wqk = (Wqkv[:, cols] * WSCALE)                    # [C, 2, NH, 128m]
    wqk = np.ascontiguousarray(
        wqk.reshape(CB2, 2, 128, 2, NH, 128)          # [j, i, p, s, h, m]
        .transpose(3, 4, 2, 0, 1, 5))                 # [s, h, p, j, i, m]
    wqkh, wqkl = _split8(wqk)

    wv = (Wqkv[:, 2 * C + hb * HD: 2 * C + (hb + NH) * HD] * WSCALE)
    wv = np.ascontiguousarray(
        wv.reshape(CB2, 2, 128, DV).transpose(0, 2, 1, 3))  # [j, p, i, w]
    wvh, wvl = _split8(wv)

    wp = np.ascontiguousarray(
        Wproj[hb * HD:(hb + NH) * HD, :].reshape(NH, 128, C)).astype(BF)

    cos2 = np.concatenate([freqs_cos.T, freqs_cos.T], 0).astype(BF)
    sin2s = np.concatenate([-freqs_sin.T, freqs_sin.T], 0).astype(BF)

    u = np.arange(2 * QTILE - 128)[None, :]
    p = np.arange(128)[:, None]
    maskbig = (p <= u - (QTILE - 128)).astype(BF)

    im = {
        "xh": xh, "xl": xl, "wqkh": wqkh, "wqkl": wqkl,
        "wvh": wvh, "wvl": wvl, "wp": wp,
        "cos2": np.ascontiguousarray(cos2), "sin2s": np.ascontiguousarray(sin2s),
        "maskbig": maskbig,
        "ones128": np.ones((128, 128), BF),
    }
    if use_bqkv:
        bqk = np.empty((128, 2 * NH), np.float32)
        for s in range(2):
            for h in range(NH):
                bqk[:, s * NH + h] = bqkv[s * C + (hb + h) * HD + _PERM]
        im["bqk"] = bqk
        im["onecol"] = np.ones((1, 128), BF)
        im["bv"] = np.ascontiguousarray(
            (bqkv[2 * C + hb * HD: 2 * C + (hb + NH) * HD] * WSCALE)[None, :]
        ).astype(BF)
    return im


_CACHE = {}


def _get_program(use_bqkv):
    key = use_bqkv
    if key not in _CACHE:
        _CACHE[key] = build_program(use_bqkv=use_bqkv)
    return _CACHE[key]


def kernel(x, freqs_cos, freqs_sin, Wqkv, bqkv, Wproj, bproj):
    x = np.asarray(x, np.float32)
    freqs_cos = np.asarray(freqs_cos, np.float32)
    freqs_sin = np.asarray(freqs_sin, np.float32)
    Wqkv = np.asarray(Wqkv, np.float32)
    bqkv = np.asarray(bqkv, np.float32)
    Wproj = np.asarray(Wproj, np.float32)
    bproj = np.asarray(bproj, np.float32)

    use_bqkv = bool(np.any(bqkv != 0))
    nc = _get_program(use_bqkv)
    in_maps = [
        shard_core(c, x, freqs_cos, freqs_sin, Wqkv, bqkv, Wproj,
                   use_bqkv=use_bqkv)
        for c in range(NCORES)
    ]
    try:
        res = run_bass_kernel_spmd(nc, in_maps, list(range(NCORES))).results
    except Exception:
        # transient device faults have been observed; retry once
        res = run_bass_kernel_spmd(nc, in_maps, list(range(NCORES))).results

    out = np.empty((B, T, C), np.float32)
    for b in range(B):
        out[b] = (res[2 * b]["out_partial"].astype(np.float32)
                  + res[2 * b + 1]["out_partial"].astype(np.float32))
    out += bproj[None, None, :]
    return out
